# revision 4
# baseline (speedup 1.0000x reference)
"""DLRM forward on 8 Trainium2 NeuronCores (Bass/Tile).

Strategy (v2):
- z has 432 feature rows (16 dense + 26*16 sparse); 432 = 8*54. Core c owns
  z-rows [48c, 48c+48) (3 whole 16-row units: unit0=dense, unit u=table u-1)
  plus a 6-row piece [384+6c, 390+6c) of tables 23..25. Each core gathers its
  units (one indirect-DMA per 128-batch chunk; the piece may span two tables
  -> two gathers with host-zero-masked columns, accum-add), transposes to
  zT-shard [54, 512], then AllGather assembles the (permuted) zT on all cores.
- Interaction + pred layer 0: out[b,o] = sum_{i,j} z_i z_j pw0[(i,j),o],
  K-sharded by i over cores in shard order (host permutes pw0 row-blocks to
  match). Per i-block: broadcast-DMA one zT row from the core's OWN shard
  bounce (B_i), DVE-multiply with the zT j-tiles -> interT chunk, then 16
  float32r matmuls (N=512) accumulate out^T in 4 PSUM banks. pw0 streams
  fp32 via HWDGE (no SWDGE descriptor-generation cost).
- Partial out^T is AllReduced (bf16) and every core redundantly computes the
  prediction MLP tail + sigmoid; core 0's output is returned.
"""

import numpy as np
import ml_dtypes

BATCH = 512
CARD = 100000
ED = 16
NCORES = 8
S = 54           # z-rows per core
ZR = 432
O = 512

BF16 = ml_dtypes.bfloat16

_state = {}


def _build_module():
    import concourse.bass as bass
    import concourse.mybir as mybir
    import concourse.tile as tile
    from concourse import bacc
    from concourse.masks import make_identity

    dt = mybir.dt
    nc = bacc.Bacc("TRN2", target_bir_lowering=False, debug=False,
                   num_devices=NCORES)

    # host-packed pw0 slices (partition-major for big DMA descriptors):
    # pw0a: full main chunks, pw0b: 48-row tails; symmetric (upper) chunks are
    # pre-added on host (W_ij + W_ji) and streamed as bf16
    # ---- per-core DRAM inputs ----
    pw0a = nc.dram_tensor("pw0a", [128, 96 * O], dt.bfloat16, kind="ExternalInput").ap()
    pw0b = nc.dram_tensor("pw0b", [48, 54 * O], dt.bfloat16, kind="ExternalInput").ap()
    embs = {}
    for nm in ("es0", "es1", "es2", "es3a", "es3b"):
        embs[nm] = nc.dram_tensor(nm, [CARD, ED], dt.bfloat16,
                                  kind="ExternalInput").ap()
    idxq = nc.dram_tensor("idxq", [128, 20], dt.int32, kind="ExternalInput").ap()
    dfT = nc.dram_tensor("dfT", [16, BATCH], dt.bfloat16, kind="ExternalInput").ap()
    dw0 = nc.dram_tensor("dw0", [16, 512], dt.bfloat16, kind="ExternalInput").ap()
    dw1 = nc.dram_tensor("dw1", [512, 256], dt.bfloat16, kind="ExternalInput").ap()
    dw2 = nc.dram_tensor("dw2", [256, 64], dt.bfloat16, kind="ExternalInput").ap()
    dw3 = nc.dram_tensor("dw3", [64, 16], dt.bfloat16, kind="ExternalInput").ap()
    db0q = nc.dram_tensor("db0q", [128, 4], dt.float32, kind="ExternalInput").ap()
    db1q = nc.dram_tensor("db1q", [128, 2], dt.float32, kind="ExternalInput").ap()
    db2q = nc.dram_tensor("db2q", [64, 1], dt.float32, kind="ExternalInput").ap()
    db3q = nc.dram_tensor("db3q", [16, 1], dt.float32, kind="ExternalInput").ap()
    pw1 = nc.dram_tensor("pw1", [512, 256], dt.bfloat16, kind="ExternalInput").ap()
    pw2 = nc.dram_tensor("pw2", [256, 1], dt.bfloat16, kind="ExternalInput").ap()
    pb0q = nc.dram_tensor("pb0q", [128, 4], dt.float32, kind="ExternalInput").ap()
    pb1q = nc.dram_tensor("pb1q", [128, 2], dt.float32, kind="ExternalInput").ap()
    pb2q = nc.dram_tensor("pb2q", [1, 1], dt.float32, kind="ExternalInput").ap()
    out_d = nc.dram_tensor("out", [1, BATCH], dt.float32, kind="ExternalOutput").ap()

    rg = [list(range(NCORES))]
    f32r = dt.float32r

    with tile.TileContext(nc) as tc:
        with tc.tile_pool(name="const", bufs=1) as cp, \
             tc.tile_pool(name="dram", bufs=1, space="DRAM") as dp:

            ag_in = dp.tile([S, BATCH], dt.bfloat16, tag="ag_in", name="ag_in")
            ag_out = dp.tile([ZR, BATCH], dt.bfloat16, tag="ag_out", name="ag_out")
            ar_in = dp.tile([O, BATCH], dt.bfloat16, tag="ar_in", name="ar_in")
            ar_out = dp.tile([O, BATCH], dt.bfloat16, tag="ar_out", name="ar_out")

            # ---- constants / small weights ----
            ident = cp.tile([128, 128], dt.bfloat16, tag="ident", name="ident")
            make_identity(nc, ident[:])
            idx_sb = cp.tile([128, 20], dt.int32, tag="idx_sb", name="idx_sb")
            nc.sync.dma_start(out=idx_sb[:], in_=idxq[:, :])
            dfT_sb = cp.tile([16, BATCH], dt.bfloat16, tag="dfT_sb", name="dfT_sb")
            nc.sync.dma_start(out=dfT_sb[:], in_=dfT[:, :])
            dw0_sb = cp.tile([16, 512], dt.bfloat16, tag="dw0_sb", name="dw0_sb")
            nc.sync.dma_start(out=dw0_sb[:], in_=dw0[:, :])
            dw1_sb = [cp.tile([128, 256], dt.bfloat16, tag=f"dw1_{k}", name=f"dw1_{k}")
                      for k in range(4)]
            for k in range(4):
                nc.sync.dma_start(out=dw1_sb[k][:], in_=dw1[k * 128:(k + 1) * 128, :])
            dw2_sb = [cp.tile([128, 64], dt.bfloat16, tag=f"dw2_{k}", name=f"dw2_{k}")
                      for k in range(2)]
            for k in range(2):
                nc.sync.dma_start(out=dw2_sb[k][:], in_=dw2[k * 128:(k + 1) * 128, :])
            dw3_sb = cp.tile([64, 16], dt.bfloat16, tag="dw3_sb", name="dw3_sb")
            nc.sync.dma_start(out=dw3_sb[:], in_=dw3[:, :])
            pw1_sb = [cp.tile([128, 256], dt.bfloat16, tag=f"pw1_{k}", name=f"pw1_{k}")
                      for k in range(4)]
            for k in range(4):
                nc.sync.dma_start(out=pw1_sb[k][:], in_=pw1[k * 128:(k + 1) * 128, :])
            pw2_sb = [cp.tile([128, 1], dt.bfloat16, tag=f"pw2_{k}", name=f"pw2_{k}")
                      for k in range(2)]
            for k in range(2):
                nc.sync.dma_start(out=pw2_sb[k][:], in_=pw2[k * 128:(k + 1) * 128, :])
            db0_sb = cp.tile([128, 4], dt.float32, tag="db0_sb", name="db0_sb")
            nc.sync.dma_start(out=db0_sb[:], in_=db0q[:, :])
            db1_sb = cp.tile([128, 2], dt.float32, tag="db1_sb", name="db1_sb")
            nc.sync.dma_start(out=db1_sb[:], in_=db1q[:, :])
            db2_sb = cp.tile([64, 1], dt.float32, tag="db2_sb", name="db2_sb")
            nc.sync.dma_start(out=db2_sb[:], in_=db2q[:, :])
            db3_sb = cp.tile([16, 1], dt.float32, tag="db3_sb", name="db3_sb")
            nc.sync.dma_start(out=db3_sb[:], in_=db3q[:, :])
            pb0_sb = cp.tile([128, 4], dt.float32, tag="pb0_sb", name="pb0_sb")
            nc.sync.dma_start(out=pb0_sb[:], in_=pb0q[:, :])
            pb1_sb = cp.tile([128, 2], dt.float32, tag="pb1_sb", name="pb1_sb")
            nc.sync.dma_start(out=pb1_sb[:], in_=pb1q[:, :])
            pb2_sb = cp.tile([1, 1], dt.float32, tag="pb2_sb", name="pb2_sb")
            nc.sync.dma_start(out=pb2_sb[:], in_=pb2q[:, :])

            # ---- gathers: slots 0-2 single table, slot 3 = two-half piece ----
            with tc.tile_pool(name="gather", bufs=1) as gp, \
                 tc.tile_pool(name="ps_g", bufs=1, space="PSUM") as pg, \
                 tc.tile_pool(name="ps_d", bufs=2, space="PSUM") as pd, \
                 tc.tile_pool(name="dmlp", bufs=1) as dm:
                zsl = []
                for s in range(4):
                    ps_s = pg.tile([16, BATCH], dt.bfloat16, tag=f"psg{s}",
                                   name=f"psg{s}")
                    for bc in range(4):
                        gt = gp.tile([128, ED], dt.bfloat16, tag=f"g{s}_{bc}",
                                     name=f"g{s}_{bc}")
                        if s < 3:
                            nc.gpsimd.indirect_dma_start(
                                out=gt[:], out_offset=None, in_=embs[f"es{s}"][:, :],
                                in_offset=bass.IndirectOffsetOnAxis(
                                    ap=idx_sb[:, s * 4 + bc:s * 4 + bc + 1], axis=0))
                        else:
                            nc.gpsimd.indirect_dma_start(
                                out=gt[:], out_offset=None, in_=embs["es3a"][:, :],
                                in_offset=bass.IndirectOffsetOnAxis(
                                    ap=idx_sb[:, 12 + bc:13 + bc], axis=0))
                            nc.gpsimd.indirect_dma_start(
                                out=gt[:], out_offset=None, in_=embs["es3b"][:, :],
                                in_offset=bass.IndirectOffsetOnAxis(
                                    ap=idx_sb[:, 16 + bc:17 + bc], axis=0),
                                compute_op=mybir.AluOpType.add)
                        nc.tensor.transpose(out=ps_s[:16, bc * 128:(bc + 1) * 128],
                                            in_=gt[:, :], identity=ident[:])
                    z_s = dm.tile([16, BATCH], dt.bfloat16, tag=f"zsl{s}",
                                  name=f"zsl{s}")
                    nc.vector.tensor_copy(out=z_s[:], in_=ps_s[:16, :])
                    zsl.append(z_s)

                # ---- dense MLP (exactly 0 on cores != 0 via zeroed dw3/db3) ----
                h1 = []
                for mc in range(4):
                    d1 = pd.tile([128, BATCH], dt.float32, tag="dscr", name="d1")
                    nc.tensor.matmul(out=d1[:], lhsT=dw0_sb[:, mc * 128:(mc + 1) * 128],
                                     rhs=dfT_sb[:], start=True, stop=True)
                    h = dm.tile([128, BATCH], dt.bfloat16, tag=f"h1_{mc}",
                                name=f"h1_{mc}")
                    nc.scalar.activation(out=h[:], in_=d1[:],
                                         func=mybir.ActivationFunctionType.Relu,
                                         bias=db0_sb[:, mc:mc + 1])
                    h1.append(h)
                h2 = []
                for mc in range(2):
                    d2 = pd.tile([128, BATCH], dt.float32, tag="dscr", name="d2")
                    for k in range(4):
                        nc.tensor.matmul(out=d2[:],
                                         lhsT=dw1_sb[k][:, mc * 128:(mc + 1) * 128],
                                         rhs=h1[k][:], start=(k == 0), stop=(k == 3))
                    h = dm.tile([128, BATCH], dt.bfloat16, tag=f"h2_{mc}",
                                name=f"h2_{mc}")
                    nc.scalar.activation(out=h[:], in_=d2[:],
                                         func=mybir.ActivationFunctionType.Relu,
                                         bias=db1_sb[:, mc:mc + 1])
                    h2.append(h)
                d3 = pd.tile([128, BATCH], dt.float32, tag="dscr", name="d3")
                for k in range(2):
                    nc.tensor.matmul(out=d3[:64, :], lhsT=dw2_sb[k][:, :],
                                     rhs=h2[k][:], start=(k == 0), stop=(k == 1))
                h3 = dm.tile([64, BATCH], dt.bfloat16, tag="h3", name="h3")
                nc.scalar.activation(out=h3[:], in_=d3[:64, :],
                                     func=mybir.ActivationFunctionType.Relu,
                                     bias=db2_sb[:, 0:1])
                d4 = pd.tile([128, BATCH], dt.float32, tag="dscr", name="d4")
                nc.tensor.matmul(out=d4[:16, :], lhsT=dw3_sb[:, :], rhs=h3[:],
                                 start=True, stop=True)
                dense_sb = dm.tile([16, BATCH], dt.bfloat16, tag="dense_sb",
                                   name="dense_sb")
                nc.vector.tensor_scalar_add(out=dense_sb[:], in0=d4[:16, :],
                                            scalar1=db3_sb[:, 0:1])
                nc.vector.tensor_add(out=zsl[0][:], in0=zsl[0][:], in1=dense_sb[:])

                for s in range(3):
                    nc.sync.dma_start(out=ag_in[16 * s:16 * s + 16, :], in_=zsl[s][:])
                nc.sync.dma_start(out=ag_in[48:54, :], in_=zsl[3][0:6, :])

            nc.gpsimd.collective_compute(
                "AllGather", mybir.AluOpType.bypass, replica_groups=rg,
                ins=[ag_in[:].opt()], outs=[ag_out[:].opt()])

            # zT j-tiles in true z-order from the permuted ag_out:
            # z-row j = 128q+16m+d (unit 8q+m owned by core m, slot q)
            #   -> ag row 54m + 16q + d;  j>=384: j=384+6c'+e -> 54c'+48+e
            zt = []
            for jc in range(3):
                t = cp.tile([128, BATCH], dt.bfloat16, tag=f"zt{jc}", name=f"zt{jc}")
                for mu in range(8):
                    nc.sync.dma_start(
                        out=t[16 * mu:16 * mu + 16, :],
                        in_=ag_out[54 * mu + 16 * jc:54 * mu + 16 * jc + 16, :])
                zt.append(t)
            t3 = cp.tile([48, BATCH], dt.bfloat16, tag="zt3", name="zt3")
            for cc in range(NCORES):
                nc.sync.dma_start(out=t3[6 * cc:6 * cc + 6, :],
                                  in_=ag_out[54 * cc + 48:54 * cc + 54, :])
            zt.append(t3)

            # ---- main loop: block-diagonal + symmetrized-upper chunks ----
            # il 0-15: q=0, 16-31: q=1, 32-47: q=2, 48-53: q=3
            with tc.tile_pool(name="wp", bufs=10) as wp, \
                 tc.tile_pool(name="bp", bufs=4) as bp, \
                 tc.tile_pool(name="ip", bufs=6) as ip, \
                 tc.tile_pool(name="ps_acc", bufs=1, space="PSUM") as pa, \
                 tc.tile_pool(name="outp", bufs=1) as op_:

                acc = [pa.tile([128, BATCH], dt.float32, tag=f"acc{oc}",
                               name=f"acc{oc}") for oc in range(4)]

                coff = 0
                for il in range(S):
                    q = min(il // 16, 3)
                    nch = 4 - q
                    nfull = 3 - q
                    wsl = wp.tile([128, 4 * O], dt.bfloat16, tag="wsl", name="wsl")
                    if nfull > 0:
                        nc.sync.dma_start(out=wsl[:, 0:nfull * O],
                                          in_=pw0a[:, coff:coff + nfull * O])
                        coff += nfull * O
                    nc.sync.dma_start(out=wsl[0:48, nfull * O:nch * O],
                                      in_=pw0b[:, il * O:(il + 1) * O])
                    b_t = bp.tile([128, BATCH], dt.bfloat16, tag="b_t", name="b_t")
                    nc.sync.dma_start(out=b_t[:],
                                      in_=ag_in[il:il + 1, :].to_broadcast([128, BATCH]))
                    for k in range(nch):
                        jcz = q + k
                        npart = 128 if jcz < 3 else 48
                        it = ip.tile([128, BATCH], dt.bfloat16, tag="it", name="it")
                        nc.vector.tensor_mul(out=it[:npart, :], in0=zt[jcz][:npart, :],
                                             in1=b_t[:npart, :])
                        for oc in range(4):
                            lhsT = wsl[:npart, k * O + oc * 128:k * O + (oc + 1) * 128]
                            nc.tensor.matmul(
                                out=acc[oc][:], lhsT=lhsT,
                                rhs=it[:npart, :],
                                start=(il == 0 and k == 0),
                                stop=(il == S - 1 and k == 0))

                for oc in range(4):
                    osb = op_.tile([128, BATCH], dt.bfloat16, tag=f"osb{oc}",
                                   name=f"osb{oc}")
                    nc.scalar.activation(out=osb[:], in_=acc[oc][:],
                                         func=mybir.ActivationFunctionType.Copy)
                    nc.sync.dma_start(out=ar_in[oc * 128:(oc + 1) * 128, :], in_=osb[:])

            nc.gpsimd.collective_compute(
                "AllReduce", mybir.AluOpType.add, replica_groups=rg,
                ins=[ar_in[:].opt()], outs=[ar_out[:].opt()])

            # ---- prediction MLP tail ----
            with tc.tile_pool(name="tail_sb", bufs=1) as ts, \
                 tc.tile_pool(name="ps_t", bufs=1, space="PSUM") as pt:
                h0 = []
                for kc in range(4):
                    r = ts.tile([128, BATCH], dt.bfloat16, tag=f"red{kc}",
                                name=f"red{kc}")
                    nc.sync.dma_start(out=r[:], in_=ar_out[kc * 128:(kc + 1) * 128, :])
                    h = ts.tile([128, BATCH], dt.bfloat16, tag=f"h0_{kc}",
                                name=f"h0_{kc}")
                    nc.scalar.activation(out=h[:], in_=r[:],
                                         func=mybir.ActivationFunctionType.Relu,
                                         bias=pb0_sb[:, kc:kc + 1])
                    h0.append(h)
                h1p = []
                for mc in range(2):
                    p1 = pt.tile([128, BATCH], dt.float32, tag=f"p1_{mc}",
                                 name=f"p1_{mc}")
                    for kc in range(4):
                        nc.tensor.matmul(out=p1[:],
                                         lhsT=pw1_sb[kc][:, mc * 128:(mc + 1) * 128],
                                         rhs=h0[kc][:], start=(kc == 0), stop=(kc == 3))
                    h = ts.tile([128, BATCH], dt.bfloat16, tag=f"h1p_{mc}",
                                name=f"h1p_{mc}")
                    nc.scalar.activation(out=h[:], in_=p1[:],
                                         func=mybir.ActivationFunctionType.Relu,
                                         bias=pb1_sb[:, mc:mc + 1])
                    h1p.append(h)
                p2 = pt.tile([1, BATCH], dt.float32, tag="p2", name="p2")
                for mc in range(2):
                    nc.tensor.matmul(out=p2[:], lhsT=pw2_sb[mc][:, :], rhs=h1p[mc][:],
                                     start=(mc == 0), stop=(mc == 1))
                res = ts.tile([1, BATCH], dt.float32, tag="res", name="res")
                nc.scalar.activation(out=res[:], in_=p2[:],
                                     func=mybir.ActivationFunctionType.Sigmoid,
                                     bias=pb2_sb[:, 0:1])
                nc.sync.dma_start(out=out_d[:, :], in_=res[:])

    nc.compile()
    return nc


def _host_prep(inputs):
    f32 = np.float32
    df = np.asarray(inputs["dense_features"], f32)
    sf = np.asarray(inputs["sparse_features"])
    emb = np.asarray(inputs["emb"], f32)
    pw0 = np.asarray(inputs["pw0"], f32)

    idx = ((sf.astype(np.int64) + 1) % CARD).astype(np.int32)   # [512, 26]
    embb = emb.astype(BF16)                                     # [26, CARD, 16]
    pw0v = pw0.reshape(ZR, ZR, O)

    dfT = np.zeros((16, BATCH), BF16)
    dfT[:13] = df.T.astype(BF16)
    dw0p = np.zeros((16, 512), f32)
    dw0p[:13] = np.asarray(inputs["dw0"], f32)

    def col(b, p):
        return np.asarray(b, f32).reshape(p, 128).T.copy()

    common = {
        "dfT": dfT,
        "dw0": dw0p.astype(BF16),
        "dw1": np.asarray(inputs["dw1"], f32).astype(BF16),
        "dw2": np.asarray(inputs["dw2"], f32).astype(BF16),
        "db0q": col(inputs["db0"], 4),
        "db1q": col(inputs["db1"], 2),
        "db2q": np.asarray(inputs["db2"], f32).reshape(64, 1).copy(),
        "pw1": np.asarray(inputs["pw1"], f32).astype(BF16),
        "pw2": np.asarray(inputs["pw2"], f32).reshape(256, 1).astype(BF16),
        "pb0q": col(inputs["pb0"], 4),
        "pb1q": col(inputs["pb1"], 2),
        "pb2q": np.asarray(inputs["pb2"], f32).reshape(1, 1).copy(),
    }
    dw3 = np.asarray(inputs["dw3"], f32).astype(BF16)
    db3 = np.asarray(inputs["db3"], f32).reshape(16, 1).astype(f32)
    zero_tab = np.zeros((CARD, ED), BF16)
    zero_idx = np.zeros(BATCH, np.int32)

    in_maps = []
    for c in range(NCORES):
        m = dict(common)
        m["dw3"] = dw3 if c == 0 else np.zeros_like(dw3)
        m["db3q"] = db3 if c == 0 else np.zeros_like(db3)

        # shard z-rows: units {c, 8+c, 16+c} (16 rows each) + piece [384+6c, +6)
        def zrow(il):
            q = min(il // 16, 3)
            if q < 3:
                return 128 * q + 16 * c + (il - 16 * q)
            return 384 + 6 * c + (il - 48)

        # partition-major packing: per il, full main chunks as [128, nfull*512]
        # (row j=128q+128k+p -> [p, k*512:...]), 48-row tails row-major.
        # Symmetric (non-diagonal) chunks get W[j,i] pre-added on host.
        pa_, pb_ = [], []
        for il in range(S):
            q = min(il // 16, 3)
            i = zrow(il)
            nfull = 3 - q
            if nfull > 0:
                blk = pw0v[i, 128 * q:128 * q + nfull * 128, :].copy()
                if nfull > 1:
                    blk[128:, :] += pw0v[128 * (q + 1):128 * (q + 1)
                                         + (nfull - 1) * 128, i, :]
                pa_.append(blk.reshape(nfull, 128, O).transpose(1, 0, 2)
                           .reshape(128, nfull * O))
            tail = pw0v[i, 384:432, :].copy()
            if q < 3:
                tail += pw0v[384:432, i, :]
            pb_.append(tail)
        m["pw0a"] = np.ascontiguousarray(np.concatenate(pa_, 1)).astype(BF16)
        m["pw0b"] = np.ascontiguousarray(np.concatenate(pb_, 1)).astype(BF16)

        idx_cols = []
        for s in range(3):
            u = [c, 8 + c, 16 + c][s]   # unit; u==0 is dense
            if u == 0:
                m[f"es{s}"] = zero_tab
                idx_cols.append(zero_idx)
            else:
                m[f"es{s}"] = np.ascontiguousarray(embb[u - 1])
                idx_cols.append(idx[:, u - 1])
        # piece: cols e=0..5 <- table 23+(6c+e)//16, dim (6c+e)%16
        ta = 23 + (6 * c) // 16
        ea = np.zeros((CARD, ED), BF16)
        eb = np.zeros((CARD, ED), BF16)
        tb = None
        for e in range(6):
            t_ = 23 + (6 * c + e) // 16
            d_ = (6 * c + e) % 16
            if t_ == ta:
                ea[:, e] = embb[t_][:, d_]
            else:
                tb = t_
                eb[:, e] = embb[t_][:, d_]
        m["es3a"] = ea
        m["es3b"] = eb
        idx_cols.append(idx[:, ta])
        idx_cols.append(idx[:, tb] if tb is not None else zero_idx)

        iq = np.zeros((128, 20), np.int32)
        for sa in range(5):
            iq[:, sa * 4:(sa + 1) * 4] = idx_cols[sa].reshape(4, 128).T
        m["idxq"] = iq
        in_maps.append(m)
    return in_maps


def kernel(**inputs):
    from concourse import bass_utils
    import os

    if "nc" not in _state:
        _state["nc"] = _build_module()
    in_maps = _host_prep(inputs)
    trace = bool(int(os.environ.get("DLRM_TRACE", "0")))
    res = bass_utils.run_bass_kernel_spmd(
        _state["nc"], in_maps, core_ids=list(range(NCORES)), trace=trace)
    _state["last_results"] = res
    return np.asarray(res.results[0]["out"], np.float32).reshape(BATCH)



# revision 6
# speedup vs baseline: 1.0599x; 1.0599x over previous
"""DLRM forward on 8 Trainium2 NeuronCores (Bass/Tile).

Strategy (v4, on top of v3's pre-symmetrized bf16 weight streaming):
- z has 432 feature rows (16 dense + 26*16 sparse); core c owns rows
  zrow(il,c): 3 whole 16-row units + a 6-row piece. Gather via indirect DMA,
  transpose, AllGather assembles permuted zT on all cores (as v2/v3).
- A tiny dummy AllReduce fires at t=0 so the one-time CC-stream init +
  cross-core barrier overlap the gather phase instead of serializing.
- Main loop: per il (0..47), 1..3 full 128-row j-chunks (symmetric side
  pre-added on host, bf16). The 54 ragged 48-row j-tail chunks (j in
  [384,432)) are packed into 21 full 128-row tiles: 3 repeating zT row
  patterns (zt3p) x per-segment broadcast multipliers (bm). Cuts matmul
  count 600 -> 468.
- b_t / bm multiplier broadcasts go on the Activation HWDGE queue to unload
  the SP sequencer; weights stream on SP.
- Partial out^T is AllReduced (bf16) in 2 chunks, pipelined with the
  prediction-MLP tail; core 0's output is returned.
"""

import numpy as np
import ml_dtypes

BATCH = 512
CARD = 100000
ED = 16
NCORES = 8
S = 54           # i-rows per core
ZR = 432
O = 512
NTT = 21         # packed tail tiles (54*48 = 2592 rows -> 21 x 128)

BF16 = ml_dtypes.bfloat16

_state = {}


def _tail_segments():
    """Per packed-tail-tile list of (il, r0, r1) multiplier segments."""
    segs = []
    for t in range(NTT):
        R0, R1 = 128 * t, min(128 * t + 128, 2592)
        s = []
        for il in range(R0 // 48, (R1 - 1) // 48 + 1):
            r0 = max(48 * il, R0) - R0
            r1 = min(48 * il + 48, R1) - R0
            s.append((il, r0, r1))
        segs.append(s)
    return segs


def _build_module():
    import concourse.bass as bass
    import concourse.mybir as mybir
    import concourse.tile as tile
    from concourse import bacc
    from concourse.masks import make_identity

    dt = mybir.dt
    nc = bacc.Bacc("TRN2", target_bir_lowering=False, debug=False,
                   num_devices=NCORES)

    # host-packed pw0 slices (partition-major for big DMA descriptors):
    # pw0a: full main chunks; pw0t: packed tails. Symmetric (upper) parts are
    # pre-added on host (W_ij + W_ji), all bf16.
    pw0a = nc.dram_tensor("pw0a", [128, 96 * O], dt.bfloat16, kind="ExternalInput").ap()
    pw0t = nc.dram_tensor("pw0t", [128, NTT * O], dt.bfloat16, kind="ExternalInput").ap()
    embs = {}
    for nm in ("es0", "es1", "es2", "es3a", "es3b"):
        embs[nm] = nc.dram_tensor(nm, [CARD, ED], dt.bfloat16,
                                  kind="ExternalInput").ap()
    idxq = nc.dram_tensor("idxq", [128, 20], dt.int32, kind="ExternalInput").ap()
    dfT = nc.dram_tensor("dfT", [16, BATCH], dt.bfloat16, kind="ExternalInput").ap()
    dw0 = nc.dram_tensor("dw0", [16, 512], dt.bfloat16, kind="ExternalInput").ap()
    dw1 = nc.dram_tensor("dw1", [512, 256], dt.bfloat16, kind="ExternalInput").ap()
    dw2 = nc.dram_tensor("dw2", [256, 64], dt.bfloat16, kind="ExternalInput").ap()
    dw3 = nc.dram_tensor("dw3", [64, 16], dt.bfloat16, kind="ExternalInput").ap()
    db0q = nc.dram_tensor("db0q", [128, 4], dt.float32, kind="ExternalInput").ap()
    db1q = nc.dram_tensor("db1q", [128, 2], dt.float32, kind="ExternalInput").ap()
    db2q = nc.dram_tensor("db2q", [64, 1], dt.float32, kind="ExternalInput").ap()
    db3q = nc.dram_tensor("db3q", [16, 1], dt.float32, kind="ExternalInput").ap()
    pw1 = nc.dram_tensor("pw1", [512, 256], dt.bfloat16, kind="ExternalInput").ap()
    pw2 = nc.dram_tensor("pw2", [256, 1], dt.bfloat16, kind="ExternalInput").ap()
    pb0q = nc.dram_tensor("pb0q", [128, 4], dt.float32, kind="ExternalInput").ap()
    pb1q = nc.dram_tensor("pb1q", [128, 2], dt.float32, kind="ExternalInput").ap()
    pb2q = nc.dram_tensor("pb2q", [1, 1], dt.float32, kind="ExternalInput").ap()
    out_d = nc.dram_tensor("out", [1, BATCH], dt.float32, kind="ExternalOutput").ap()

    rg = [list(range(NCORES))]
    segs = _tail_segments()

    with tile.TileContext(nc) as tc:
        with tc.tile_pool(name="const", bufs=1) as cp, \
             tc.tile_pool(name="dram", bufs=1, space="DRAM") as dp:

            ag_in = dp.tile([S, BATCH], dt.bfloat16, tag="ag_in", name="ag_in")
            ag_out = dp.tile([ZR, BATCH], dt.bfloat16, tag="ag_out", name="ag_out")
            ar_in = dp.tile([O, BATCH], dt.bfloat16, tag="ar_in", name="ar_in")
            ar_out = dp.tile([O, BATCH], dt.bfloat16, tag="ar_out", name="ar_out")
            dum_i = dp.tile([1, 4], dt.float32, tag="dum_i", name="dum_i")
            dum_o = dp.tile([1, 4], dt.float32, tag="dum_o", name="dum_o")

            # warm the CC stream / absorb the first-collective barrier early
            # (collectives cannot read IO tensors -> bounce via SBUF)
            tiny = cp.tile([1, 4], dt.float32, tag="tiny", name="tiny")
            nc.sync.dma_start(out=tiny[:], in_=db0q[0:1, 0:4])
            nc.sync.dma_start(out=dum_i[:], in_=tiny[:])
            nc.gpsimd.collective_compute(
                "AllReduce", mybir.AluOpType.add, replica_groups=rg,
                ins=[dum_i[:].opt()], outs=[dum_o[:].opt()])

            # ---- constants / small weights ----
            ident = cp.tile([128, 128], dt.bfloat16, tag="ident", name="ident")
            make_identity(nc, ident[:])
            idx_sb = cp.tile([128, 20], dt.int32, tag="idx_sb", name="idx_sb")
            nc.sync.dma_start(out=idx_sb[:], in_=idxq[:, :])
            dfT_sb = cp.tile([16, BATCH], dt.bfloat16, tag="dfT_sb", name="dfT_sb")
            nc.sync.dma_start(out=dfT_sb[:], in_=dfT[:, :])
            dw0_sb = cp.tile([16, 512], dt.bfloat16, tag="dw0_sb", name="dw0_sb")
            nc.sync.dma_start(out=dw0_sb[:], in_=dw0[:, :])
            dw1_sb = [cp.tile([128, 256], dt.bfloat16, tag=f"dw1_{k}", name=f"dw1_{k}")
                      for k in range(4)]
            for k in range(4):
                nc.sync.dma_start(out=dw1_sb[k][:], in_=dw1[k * 128:(k + 1) * 128, :])
            dw2_sb = [cp.tile([128, 64], dt.bfloat16, tag=f"dw2_{k}", name=f"dw2_{k}")
                      for k in range(2)]
            for k in range(2):
                nc.sync.dma_start(out=dw2_sb[k][:], in_=dw2[k * 128:(k + 1) * 128, :])
            dw3_sb = cp.tile([64, 16], dt.bfloat16, tag="dw3_sb", name="dw3_sb")
            nc.sync.dma_start(out=dw3_sb[:], in_=dw3[:, :])
            pw1_sb = [cp.tile([128, 256], dt.bfloat16, tag=f"pw1_{k}", name=f"pw1_{k}")
                      for k in range(4)]
            for k in range(4):
                nc.sync.dma_start(out=pw1_sb[k][:], in_=pw1[k * 128:(k + 1) * 128, :])
            pw2_sb = [cp.tile([128, 1], dt.bfloat16, tag=f"pw2_{k}", name=f"pw2_{k}")
                      for k in range(2)]
            for k in range(2):
                nc.sync.dma_start(out=pw2_sb[k][:], in_=pw2[k * 128:(k + 1) * 128, :])
            db0_sb = cp.tile([128, 4], dt.float32, tag="db0_sb", name="db0_sb")
            nc.sync.dma_start(out=db0_sb[:], in_=db0q[:, :])
            db1_sb = cp.tile([128, 2], dt.float32, tag="db1_sb", name="db1_sb")
            nc.sync.dma_start(out=db1_sb[:], in_=db1q[:, :])
            db2_sb = cp.tile([64, 1], dt.float32, tag="db2_sb", name="db2_sb")
            nc.sync.dma_start(out=db2_sb[:], in_=db2q[:, :])
            db3_sb = cp.tile([16, 1], dt.float32, tag="db3_sb", name="db3_sb")
            nc.sync.dma_start(out=db3_sb[:], in_=db3q[:, :])
            pb0_sb = cp.tile([128, 4], dt.float32, tag="pb0_sb", name="pb0_sb")
            nc.sync.dma_start(out=pb0_sb[:], in_=pb0q[:, :])
            pb1_sb = cp.tile([128, 2], dt.float32, tag="pb1_sb", name="pb1_sb")
            nc.sync.dma_start(out=pb1_sb[:], in_=pb1q[:, :])
            pb2_sb = cp.tile([1, 1], dt.float32, tag="pb2_sb", name="pb2_sb")
            nc.sync.dma_start(out=pb2_sb[:], in_=pb2q[:, :])

            # ---- gathers: slots 0-2 single table, slot 3 = two-half piece ----
            with tc.tile_pool(name="gather", bufs=1) as gp, \
                 tc.tile_pool(name="ps_g", bufs=1, space="PSUM") as pg, \
                 tc.tile_pool(name="ps_d", bufs=2, space="PSUM") as pd, \
                 tc.tile_pool(name="dmlp", bufs=1) as dm:
                zsl = []
                for s in range(4):
                    ps_s = pg.tile([16, BATCH], dt.bfloat16, tag=f"psg{s}",
                                   name=f"psg{s}")
                    for bc in range(4):
                        gt = gp.tile([128, ED], dt.bfloat16, tag=f"g{s}_{bc}",
                                     name=f"g{s}_{bc}")
                        if s < 3:
                            nc.gpsimd.indirect_dma_start(
                                out=gt[:], out_offset=None, in_=embs[f"es{s}"][:, :],
                                in_offset=bass.IndirectOffsetOnAxis(
                                    ap=idx_sb[:, s * 4 + bc:s * 4 + bc + 1], axis=0))
                        else:
                            nc.gpsimd.indirect_dma_start(
                                out=gt[:], out_offset=None, in_=embs["es3a"][:, :],
                                in_offset=bass.IndirectOffsetOnAxis(
                                    ap=idx_sb[:, 12 + bc:13 + bc], axis=0))
                            nc.gpsimd.indirect_dma_start(
                                out=gt[:], out_offset=None, in_=embs["es3b"][:, :],
                                in_offset=bass.IndirectOffsetOnAxis(
                                    ap=idx_sb[:, 16 + bc:17 + bc], axis=0),
                                compute_op=mybir.AluOpType.add)
                        nc.tensor.transpose(out=ps_s[:16, bc * 128:(bc + 1) * 128],
                                            in_=gt[:, :], identity=ident[:])
                    z_s = dm.tile([16, BATCH], dt.bfloat16, tag=f"zsl{s}",
                                  name=f"zsl{s}")
                    nc.vector.tensor_copy(out=z_s[:], in_=ps_s[:16, :])
                    zsl.append(z_s)

                # ---- dense MLP (exactly 0 on cores != 0 via zeroed dw3/db3) ----
                h1 = []
                for mc in range(4):
                    d1 = pd.tile([128, BATCH], dt.float32, tag="dscr", name="d1")
                    nc.tensor.matmul(out=d1[:], lhsT=dw0_sb[:, mc * 128:(mc + 1) * 128],
                                     rhs=dfT_sb[:], start=True, stop=True)
                    h = dm.tile([128, BATCH], dt.bfloat16, tag=f"h1_{mc}",
                                name=f"h1_{mc}")
                    nc.scalar.activation(out=h[:], in_=d1[:],
                                         func=mybir.ActivationFunctionType.Relu,
                                         bias=db0_sb[:, mc:mc + 1])
                    h1.append(h)
                h2 = []
                for mc in range(2):
                    d2 = pd.tile([128, BATCH], dt.float32, tag="dscr", name="d2")
                    for k in range(4):
                        nc.tensor.matmul(out=d2[:],
                                         lhsT=dw1_sb[k][:, mc * 128:(mc + 1) * 128],
                                         rhs=h1[k][:], start=(k == 0), stop=(k == 3))
                    h = dm.tile([128, BATCH], dt.bfloat16, tag=f"h2_{mc}",
                                name=f"h2_{mc}")
                    nc.scalar.activation(out=h[:], in_=d2[:],
                                         func=mybir.ActivationFunctionType.Relu,
                                         bias=db1_sb[:, mc:mc + 1])
                    h2.append(h)
                d3 = pd.tile([128, BATCH], dt.float32, tag="dscr", name="d3")
                for k in range(2):
                    nc.tensor.matmul(out=d3[:64, :], lhsT=dw2_sb[k][:, :],
                                     rhs=h2[k][:], start=(k == 0), stop=(k == 1))
                h3 = dm.tile([64, BATCH], dt.bfloat16, tag="h3", name="h3")
                nc.scalar.activation(out=h3[:], in_=d3[:64, :],
                                     func=mybir.ActivationFunctionType.Relu,
                                     bias=db2_sb[:, 0:1])
                d4 = pd.tile([128, BATCH], dt.float32, tag="dscr", name="d4")
                nc.tensor.matmul(out=d4[:16, :], lhsT=dw3_sb[:, :], rhs=h3[:],
                                 start=True, stop=True)
                dense_sb = dm.tile([16, BATCH], dt.bfloat16, tag="dense_sb",
                                   name="dense_sb")
                nc.vector.tensor_scalar_add(out=dense_sb[:], in0=d4[:16, :],
                                            scalar1=db3_sb[:, 0:1])
                nc.vector.tensor_add(out=zsl[0][:], in0=zsl[0][:], in1=dense_sb[:])

                for s in range(3):
                    nc.sync.dma_start(out=ag_in[16 * s:16 * s + 16, :], in_=zsl[s][:])
                nc.sync.dma_start(out=ag_in[48:54, :], in_=zsl[3][0:6, :])

            nc.gpsimd.collective_compute(
                "AllGather", mybir.AluOpType.bypass, replica_groups=rg,
                ins=[ag_in[:].opt()], outs=[ag_out[:].opt()])

            # zT j-tiles in true z-order from the permuted ag_out:
            # z-row j = 128q+16m+d (unit 8q+m owned by core m, slot q)
            #   -> ag row 54m + 16q + d;  j>=384: j=384+6c'+e -> 54c'+48+e
            zt = []
            for jc in range(3):
                t = cp.tile([128, BATCH], dt.bfloat16, tag=f"zt{jc}", name=f"zt{jc}")
                for mu in range(8):
                    nc.sync.dma_start(
                        out=t[16 * mu:16 * mu + 16, :],
                        in_=ag_out[54 * mu + 16 * jc:54 * mu + 16 * jc + 16, :])
                zt.append(t)
            t3 = cp.tile([48, BATCH], dt.bfloat16, tag="zt3", name="zt3")
            for cc in range(NCORES):
                nc.sync.dma_start(out=t3[6 * cc:6 * cc + 6, :],
                                  in_=ag_out[54 * cc + 48:54 * cc + 54, :])

            # repeating zT row patterns for the packed tail tiles:
            # tile t row p holds tail row d = (128*t + p) mod 48, t mod 3 cases
            zt3p = [cp.tile([128, BATCH], dt.bfloat16, tag=f"zt3p{i}",
                            name=f"zt3p{i}") for i in range(3)]
            for dst, src0, src1 in ((0, 0, 48), (0, 48, 96), (0, 96, 128),
                                    (1, 16, 64), (1, 64, 112),
                                    (2, 32, 80), (2, 80, 128)):
                nc.sync.dma_start(out=zt3p[dst][src0:src1, :],
                                  in_=t3[0:src1 - src0, :])
            nc.sync.dma_start(out=zt3p[1][0:16, :], in_=t3[32:48, :])
            nc.sync.dma_start(out=zt3p[1][112:128, :], in_=t3[0:16, :])
            nc.sync.dma_start(out=zt3p[2][0:32, :], in_=t3[16:48, :])

            # ---- main loop: full 128-row chunks (diag + symmetrized upper) ----
            with tc.tile_pool(name="wp", bufs=10) as wp, \
                 tc.tile_pool(name="wq", bufs=3) as wq, \
                 tc.tile_pool(name="bp", bufs=6) as bp, \
                 tc.tile_pool(name="ip", bufs=6) as ip, \
                 tc.tile_pool(name="ps_acc", bufs=1, space="PSUM") as pa, \
                 tc.tile_pool(name="outp", bufs=1) as op_:

                acc = [pa.tile([128, BATCH], dt.float32, tag=f"acc{oc}",
                               name=f"acc{oc}") for oc in range(4)]

                coff = 0
                for il in range(48):
                    q = il // 16
                    nfull = 3 - q
                    wsl = wp.tile([128, 3 * O], dt.bfloat16, tag="wsl", name="wsl")
                    nc.sync.dma_start(out=wsl[:, 0:nfull * O],
                                      in_=pw0a[:, coff:coff + nfull * O])
                    coff += nfull * O
                    b_t = bp.tile([128, BATCH], dt.bfloat16, tag="b_t", name="b_t")
                    nc.scalar.dma_start(
                        out=b_t[:],
                        in_=ag_in[il:il + 1, :].to_broadcast([128, BATCH]))
                    for k in range(nfull):
                        jcz = q + k
                        it = ip.tile([128, BATCH], dt.bfloat16, tag="it", name="it")
                        nc.vector.tensor_mul(out=it[:], in0=zt[jcz][:],
                                             in1=b_t[:])
                        for oc in range(4):
                            lhsT = wsl[:, k * O + oc * 128:k * O + (oc + 1) * 128]
                            nc.tensor.matmul(
                                out=acc[oc][:], lhsT=lhsT, rhs=it[:],
                                start=(il == 0 and k == 0), stop=False)

                # ---- packed tail tiles ----
                for t in range(NTT):
                    g = t // 3
                    if t % 3 == 0:
                        wt = wq.tile([128, 3 * O], dt.bfloat16, tag="wt", name="wt")
                        nc.sync.dma_start(out=wt[:],
                                          in_=pw0t[:, g * 3 * O:(g + 1) * 3 * O])
                    npart = 128 if t < NTT - 1 else 32
                    bm = bp.tile([128, BATCH], dt.bfloat16, tag="bm", name="bm")
                    for (il, r0, r1) in segs[t]:
                        if r0 >= npart:
                            continue
                        nc.scalar.dma_start(
                            out=bm[r0:min(r1, npart), :],
                            in_=ag_in[il:il + 1, :].to_broadcast(
                                [min(r1, npart) - r0, BATCH]))
                    itp = ip.tile([128, BATCH], dt.bfloat16, tag="it", name="itp")
                    nc.vector.tensor_mul(out=itp[:npart, :],
                                         in0=zt3p[t % 3][:npart, :],
                                         in1=bm[:npart, :])
                    for oc in range(4):
                        lhsT = wt[:npart, (t % 3) * O + oc * 128:
                                  (t % 3) * O + (oc + 1) * 128]
                        nc.tensor.matmul(
                            out=acc[oc][:], lhsT=lhsT, rhs=itp[:npart, :],
                            start=False, stop=(t == NTT - 1))

                for oc in range(4):
                    osb = op_.tile([128, BATCH], dt.bfloat16, tag=f"osb{oc}",
                                   name=f"osb{oc}")
                    nc.scalar.activation(out=osb[:], in_=acc[oc][:],
                                         func=mybir.ActivationFunctionType.Copy)
                    nc.sync.dma_start(out=ar_in[oc * 128:(oc + 1) * 128, :], in_=osb[:])

            # chunked AllReduce, pipelined with the prediction tail
            nc.gpsimd.collective_compute(
                "AllReduce", mybir.AluOpType.add, replica_groups=rg,
                ins=[ar_in[0:256, :].opt()], outs=[ar_out[0:256, :].opt()])
            nc.gpsimd.collective_compute(
                "AllReduce", mybir.AluOpType.add, replica_groups=rg,
                ins=[ar_in[256:512, :].opt()], outs=[ar_out[256:512, :].opt()])

            # ---- prediction MLP tail ----
            with tc.tile_pool(name="tail_sb", bufs=1) as ts, \
                 tc.tile_pool(name="ps_t", bufs=1, space="PSUM") as pt:
                h0 = []
                for kc in range(4):
                    r = ts.tile([128, BATCH], dt.bfloat16, tag=f"red{kc}",
                                name=f"red{kc}")
                    nc.sync.dma_start(out=r[:], in_=ar_out[kc * 128:(kc + 1) * 128, :])
                    h = ts.tile([128, BATCH], dt.bfloat16, tag=f"h0_{kc}",
                                name=f"h0_{kc}")
                    nc.scalar.activation(out=h[:], in_=r[:],
                                         func=mybir.ActivationFunctionType.Relu,
                                         bias=pb0_sb[:, kc:kc + 1])
                    h0.append(h)
                h1p = []
                for mc in range(2):
                    p1 = pt.tile([128, BATCH], dt.float32, tag=f"p1_{mc}",
                                 name=f"p1_{mc}")
                    for kc in range(4):
                        nc.tensor.matmul(out=p1[:],
                                         lhsT=pw1_sb[kc][:, mc * 128:(mc + 1) * 128],
                                         rhs=h0[kc][:], start=(kc == 0), stop=(kc == 3))
                    h = ts.tile([128, BATCH], dt.bfloat16, tag=f"h1p_{mc}",
                                name=f"h1p_{mc}")
                    nc.scalar.activation(out=h[:], in_=p1[:],
                                         func=mybir.ActivationFunctionType.Relu,
                                         bias=pb1_sb[:, mc:mc + 1])
                    h1p.append(h)
                p2 = pt.tile([1, BATCH], dt.float32, tag="p2", name="p2")
                for mc in range(2):
                    nc.tensor.matmul(out=p2[:], lhsT=pw2_sb[mc][:, :], rhs=h1p[mc][:],
                                     start=(mc == 0), stop=(mc == 1))
                res = ts.tile([1, BATCH], dt.float32, tag="res", name="res")
                nc.scalar.activation(out=res[:], in_=p2[:],
                                     func=mybir.ActivationFunctionType.Sigmoid,
                                     bias=pb2_sb[:, 0:1])
                nc.sync.dma_start(out=out_d[:, :], in_=res[:])

    nc.compile()
    return nc


def _host_prep(inputs):
    f32 = np.float32
    df = np.asarray(inputs["dense_features"], f32)
    sf = np.asarray(inputs["sparse_features"])
    emb = np.asarray(inputs["emb"], f32)
    pw0 = np.asarray(inputs["pw0"], f32)

    idx = ((sf.astype(np.int64) + 1) % CARD).astype(np.int32)   # [512, 26]
    embb = emb.astype(BF16)                                     # [26, CARD, 16]
    pw0v = pw0.reshape(ZR, ZR, O)

    dfT = np.zeros((16, BATCH), BF16)
    dfT[:13] = df.T.astype(BF16)
    dw0p = np.zeros((16, 512), f32)
    dw0p[:13] = np.asarray(inputs["dw0"], f32)

    def col(b, p):
        return np.asarray(b, f32).reshape(p, 128).T.copy()

    common = {
        "dfT": dfT,
        "dw0": dw0p.astype(BF16),
        "dw1": np.asarray(inputs["dw1"], f32).astype(BF16),
        "dw2": np.asarray(inputs["dw2"], f32).astype(BF16),
        "db0q": col(inputs["db0"], 4),
        "db1q": col(inputs["db1"], 2),
        "db2q": np.asarray(inputs["db2"], f32).reshape(64, 1).copy(),
        "pw1": np.asarray(inputs["pw1"], f32).astype(BF16),
        "pw2": np.asarray(inputs["pw2"], f32).reshape(256, 1).astype(BF16),
        "pb0q": col(inputs["pb0"], 4),
        "pb1q": col(inputs["pb1"], 2),
        "pb2q": np.asarray(inputs["pb2"], f32).reshape(1, 1).copy(),
    }
    dw3 = np.asarray(inputs["dw3"], f32).astype(BF16)
    db3 = np.asarray(inputs["db3"], f32).reshape(16, 1).astype(f32)
    zero_tab = np.zeros((CARD, ED), BF16)
    zero_idx = np.zeros(BATCH, np.int32)

    in_maps = []
    for c in range(NCORES):
        m = dict(common)
        m["dw3"] = dw3 if c == 0 else np.zeros_like(dw3)
        m["db3q"] = db3 if c == 0 else np.zeros_like(db3)

        # shard z-rows: units {c, 8+c, 16+c} (16 rows each) + piece [384+6c, +6)
        def zrow(il):
            q = min(il // 16, 3)
            if q < 3:
                return 128 * q + 16 * c + (il - 16 * q)
            return 384 + 6 * c + (il - 48)

        # partition-major packing: per il (0..47), full chunks [128, nfull*512]
        # (row j=128(q+k)+p -> [p, k*512:...]); diag chunk (k=0) is the raw
        # block, k>=1 chunks get W[j,i] pre-added.
        pa_ = []
        for il in range(48):
            q = il // 16
            nfull = 3 - q
            i = zrow(il)
            blk = pw0v[i, 128 * q:128 * q + nfull * 128, :].copy()
            if nfull > 1:
                blk[128:, :] += pw0v[128 * (q + 1):128 * (q + 1)
                                     + (nfull - 1) * 128, i, :]
            pa_.append(blk.reshape(nfull, 128, O).transpose(1, 0, 2)
                       .reshape(128, nfull * O))
        m["pw0a"] = np.ascontiguousarray(np.concatenate(pa_, 1)).astype(BF16)

        # packed tails: flat row R = 48*il + (j-384), R in [0, 2592), padded
        # to 21*128; tile t col-block holds rows R = 128t..128t+127.
        tailW = np.zeros((NTT * 128, O), f32)
        for il in range(S):
            q = min(il // 16, 3)
            i = zrow(il)
            blk = pw0v[i, 384:432, :].copy()
            if q < 3:
                blk += pw0v[384:432, i, :]
            tailW[48 * il:48 * il + 48] = blk
        m["pw0t"] = np.ascontiguousarray(
            tailW.reshape(NTT, 128, O).transpose(1, 0, 2)
            .reshape(128, NTT * O)).astype(BF16)

        idx_cols = []
        for s in range(3):
            u = [c, 8 + c, 16 + c][s]   # unit; u==0 is dense
            if u == 0:
                m[f"es{s}"] = zero_tab
                idx_cols.append(zero_idx)
            else:
                m[f"es{s}"] = np.ascontiguousarray(embb[u - 1])
                idx_cols.append(idx[:, u - 1])
        # piece: cols e=0..5 <- table 23+(6c+e)//16, dim (6c+e)%16
        ta = 23 + (6 * c) // 16
        ea = np.zeros((CARD, ED), BF16)
        eb = np.zeros((CARD, ED), BF16)
        tb = None
        for e in range(6):
            t_ = 23 + (6 * c + e) // 16
            d_ = (6 * c + e) % 16
            if t_ == ta:
                ea[:, e] = embb[t_][:, d_]
            else:
                tb = t_
                eb[:, e] = embb[t_][:, d_]
        m["es3a"] = ea
        m["es3b"] = eb
        idx_cols.append(idx[:, ta])
        idx_cols.append(idx[:, tb] if tb is not None else zero_idx)

        iq = np.zeros((128, 20), np.int32)
        for sa in range(5):
            iq[:, sa * 4:(sa + 1) * 4] = idx_cols[sa].reshape(4, 128).T
        m["idxq"] = iq
        in_maps.append(m)
    return in_maps


def kernel(**inputs):
    from concourse import bass_utils
    import os

    if "nc" not in _state:
        _state["nc"] = _build_module()
    in_maps = _host_prep(inputs)
    trace = bool(int(os.environ.get("DLRM_TRACE", "0")))
    res = bass_utils.run_bass_kernel_spmd(
        _state["nc"], in_maps, core_ids=list(range(NCORES)), trace=trace)
    _state["last_results"] = res
    return np.asarray(res.results[0]["out"], np.float32).reshape(BATCH)
